# revision 16
# baseline (speedup 1.0000x reference)
"""Trainium2 Bass kernel for nn_MultiHeadAttention (B=4, S=2048, D=1024, H=16).

Sharding: 8 cores, core c handles batch b=c//2 and query-row half qh=c%2
(1024 query rows), with all 16 heads and the full 2048-key context for
that batch.  No collectives: each core produces a disjoint [1024, 1024]
slab of the output.

v2 dataflow (everything SBUF-resident in bf16 after the PSUM stage):
  - x.T obtained via HWDGE DMA-transpose (bf16, 16x128 xbar tiles), not PE
    transposes: DMA x fp32 -> SBUF, cast fp32->bf16 (ACT early / DVE late),
    dma_start_transpose -> x.T[p, it, tok] = x[tok, it*128+p].
  - Projections in bf16 (weights cast on DVE at load), tb-major.  V first
    (so attention AV can drain as soon as scores flow), then K, then Q.
    Evictions add biases on DVE and write bf16 into resident tiles:
      kt_sb/qt_sb [dim-in-pair(128), pair, token] feature-major,
      vt_sb [key-in-tile(128), kt, head, 65] token-major with a ones column
      (the ones column makes AV also produce the softmax denominator).
  - Attention per (pair, qb): scores via 2 row-packed matmuls (2 heads at
    base partitions 0/64 run concurrently on the PE), exp on ACT with the
    1/sqrt(64) scale folded in (fp32 PSUM -> bf16 SBUF), AV accumulates
    [65, 512] over the 16 key tiles, normalize = DVE reciprocal of row 64 +
    GPSIMD partition-broadcast + DVE multiply -> xtn_sb bf16.
    The exp stream is the single largest engine load (~270us on ACT); the
    emission order interleaves scores/exp bursts into the K-projection
    phase (exp-ahead buffered in the pe pool) so ACT never starves.
  - Output projection in bf16, split by query half so the first half
    overlaps the second half's attention sweeps.
"""

import os
import sys

import numpy as np

sys.path.insert(0, "/opt/trn_rl_repo")

import concourse.bass as bass  # noqa: E402
import concourse.tile as tile  # noqa: E402
from concourse import bacc, mybir  # noqa: E402
from concourse.bass_utils import run_bass_kernel_spmd  # noqa: E402

B, S, D, H = 4, 2048, 1024, 16
HD = D // H          # 64
P = 128
SQ = S // 2          # query rows per core
SK = S               # key rows per core
NIT = D // P         # 8 input-feature tiles
NOT = D // P         # 8 output-feature tiles
KT = SK // P         # 16 key-token tiles
NP = H // 2          # 8 head pairs
VW = HD + 1          # 65: head slice of V plus ones column
TBK = SK // 512      # 4 token blocks (K/V)
TBQ = SQ // 512      # 2 token blocks (Q)

F32 = mybir.dt.float32
BF16 = mybir.dt.bfloat16
EXP = mybir.ActivationFunctionType.Exp
ADD = mybir.AluOpType.add
MULT = mybir.AluOpType.mult

_CACHE: dict = {}


def _emit(tc, io):
    nc = tc.nc

    persist = tc.alloc_tile_pool(name="persist", bufs=1)
    consts = tc.alloc_tile_pool(name="consts", bufs=1)

    # resident bf16 tensors
    kt_sb = persist.tile([P, NP, SK], BF16, tag="ktr")
    qt_sb = persist.tile([P, NP, SQ], BF16, tag="qtr")
    vt_sb = persist.tile([P, KT, H, VW], BF16, tag="vtr")
    xtn_sb = persist.tile([P, NP, SQ], BF16, tag="xtn")

    # biases in per-partition layout: b*[a*128 + p] = tile[p, a]
    bqt = consts.tile([P, NOT], F32, tag="bqt")
    nc.sync.dma_start(out=bqt[:], in_=io["bq"].rearrange("(a p) -> p a", p=P))
    bkt = consts.tile([P, NOT], F32, tag="bkt")
    nc.sync.dma_start(out=bkt[:], in_=io["bk"].rearrange("(a p) -> p a", p=P))
    bv_row = consts.tile([1, D], F32, tag="bvr")
    nc.sync.dma_start(out=bv_row[:], in_=io["bv"].rearrange("(a d) -> a d", a=1))
    bo_row = consts.tile([1, D], F32, tag="bor")
    nc.sync.dma_start(out=bo_row[:], in_=io["bo"].rearrange("(a d) -> a d", a=1))
    bv_bcast = consts.tile([P, D], F32, tag="bvb")
    nc.gpsimd.partition_broadcast(bv_bcast[:], bv_row[0:1, :])
    bo_bcast = consts.tile([P, D], F32, tag="bob")
    nc.gpsimd.partition_broadcast(bo_bcast[:], bo_row[0:1, :])
    # ones column of the augmented V, written once
    nc.vector.memset(vt_sb[:, :, :, HD : HD + 1], 1.0)

    # long-lived pools on the left stack; projection-phase pools on the
    # right stack so they can be released (LIFO) mid-kernel.
    w_pool = tc.alloc_tile_pool(name="w", bufs=2)
    # transient exp tiles (consumed by the next AV) vs exp-ahead tiles
    # (held across the projection phase) live in separate pools so the
    # ahead tiles can't wedge the transient ring.
    pe_pool = tc.alloc_tile_pool(name="pe", bufs=4)
    ahead_pool = tc.alloc_tile_pool(name="ahead", bufs=6)
    rb_pool = tc.alloc_tile_pool(name="rb", bufs=2)
    rcp_pool = tc.alloc_tile_pool(name="rcp", bufs=2)
    xrow_pool = tc.alloc_tile_pool(name="xrow", bufs=2, side="right")
    xbf_pool = tc.alloc_tile_pool(name="xbf", bufs=2, side="right")
    xt_pool = tc.alloc_tile_pool(name="xt", bufs=2, side="right")
    wst_pool = tc.alloc_tile_pool(name="wst", bufs=1, side="right")
    # PSUM budget (8 banks): proj 2x[128,512]=2, scores 2x[128,1024]=4,
    # x accumulator 1x[65,1024]=2.  After proj_ps releases, o_ps takes
    # its 2 banks for the output projection that overlaps the qb1 sweeps.
    s_ps = tc.alloc_tile_pool(name="s_ps", bufs=2, space="PSUM")
    x_ps = tc.alloc_tile_pool(name="x_ps", bufs=1, space="PSUM")
    proj_ps = tc.alloc_tile_pool(name="proj_ps", bufs=2, space="PSUM")

    def load_weight(which):
        """DMA fp32 weight row-slices, cast to bf16 on DVE."""
        w_bf = w_pool.tile([P, NIT, D], BF16, tag="w", name=f"w_{which}")
        for it in range(NIT):
            st = wst_pool.tile([P, D], F32, tag="wst", name=f"ws_{which}_{it}")
            nc.sync.dma_start(out=st[:], in_=io[which][it * P : (it + 1) * P, :])
            nc.vector.tensor_copy(w_bf[:, it], st[:])
        return w_bf

    def stage_block(x_ap, g, xt_tile, j, cast):
        """One 128-token block: DMA fp32, cast to bf16, DMA-transpose."""
        xrow = xrow_pool.tile([P, D], F32, tag="xrow", name=f"xr_{g}")
        nc.sync.dma_start(out=xrow[:], in_=x_ap[g * P : (g + 1) * P, :])
        xbf = xbf_pool.tile([P, D], BF16, tag="xbf", name=f"xb_{g}")
        if cast == "act":
            nc.scalar.copy(xbf[:], xrow[:])
        elif cast == "pool":
            nc.gpsimd.tensor_copy(xbf[:], xrow[:])
        else:
            nc.vector.tensor_copy(xbf[:], xrow[:])
        nc.sync.dma_start_transpose(out=xt_tile[:, :, j * P : (j + 1) * P], in_=xbf[:])

    def stage_tb(x_ap, name, tb, cast):
        xt_t = xt_pool.tile([P, NIT, 512], BF16, tag="xt", name=f"xt_{name}_{tb}")
        for j in range(4):
            stage_block(x_ap, tb * 4 + j, xt_t, j, cast)
        return xt_t

    def proj_qk_ot(w_bf, xt_t, tb, ot, dst, bias):
        ps = proj_ps.tile([P, 512], F32, tag="pj", name=f"p_{tb}_{ot}")
        for it in range(NIT):
            nc.tensor.matmul(
                ps[:],
                w_bf[:, it, ot * P : (ot + 1) * P],
                xt_t[:, it],
                start=(it == 0),
                stop=(it == NIT - 1),
            )
        nc.vector.tensor_scalar_add(
            dst[:, ot, tb * 512 : (tb + 1) * 512], ps[:], bias[:, ot : ot + 1]
        )

    def proj_qk_tb(w_bf, xt_t, tb, dst, bias):
        """Q/K projection for one 512-token block; evict bf16 + bias (DVE)."""
        for ot in range(NOT):
            proj_qk_ot(w_bf, xt_t, tb, ot, dst, bias)

    def proj_v_tb(w_bf, xt_t, tb):
        """V projection for one 512-key block, token-major eviction + bias."""
        for ts in range(4):
            kt = tb * 4 + ts
            for ob in range(2):
                ps = proj_ps.tile([P, 512], F32, tag="pj", name=f"v_{kt}_{ob}")
                for it in range(NIT):
                    nc.tensor.matmul(
                        ps[:],
                        xt_t[:, it, ts * P : (ts + 1) * P],
                        w_bf[:, it, ob * 512 : (ob + 1) * 512],
                        start=(it == 0),
                        stop=(it == NIT - 1),
                    )
                nc.vector.tensor_tensor(
                    vt_sb[:, kt, ob * NOT : (ob + 1) * NOT, 0:HD],
                    ps[:].rearrange("p (h c) -> p h c", c=HD),
                    bv_bcast[:, ob * 512 : (ob + 1) * 512].rearrange(
                        "p (h c) -> p h c", c=HD
                    ),
                    op=ADD,
                )

    pe_buf = {}

    def scores_exp(pr, qb, kt, pool):
        """Row-packed scores for both heads of the pair + exp -> bf16 P."""
        sp = s_ps.tile([P, 1024], F32, tag="sp", name=f"sp_{pr}_{qb}_{kt}")
        for h2 in range(2):
            nc.tensor.matmul(
                sp[:, h2 * 512 : (h2 + 1) * 512],
                kt_sb[h2 * HD : (h2 + 1) * HD, pr, kt * P : (kt + 1) * P],
                qt_sb[h2 * HD : (h2 + 1) * HD, pr, qb * 512 : (qb + 1) * 512],
            )
        pet = pool.tile([P, 1024], BF16, tag="pe", name=f"pe_{pr}_{qb}_{kt}")
        nc.scalar.activation(pet[:], sp[:], EXP, scale=1.0 / 8.0)
        return pet

    def exp_ahead(pr, qb, kts):
        for kt in kts:
            pe_buf[(pr, qb, kt)] = scores_exp(pr, qb, kt, ahead_pool)

    def sweep_iter(pr, qb, kt, xs):
        pet = pe_buf.pop((pr, qb, kt), None)
        if pet is None:
            pet = scores_exp(pr, qb, kt, pe_pool)
        for h2 in range(2):
            nc.tensor.matmul(
                xs[:, h2 * 512 : (h2 + 1) * 512],
                vt_sb[:, kt, 2 * pr + h2, :],
                pet[:, h2 * 512 : (h2 + 1) * 512],
                start=(kt == 0),
                stop=(kt == KT - 1),
            )

    def normalize(pr, qb, xs):
        for h2 in range(2):
            xsl = xs[:, h2 * 512 : (h2 + 1) * 512]
            rcp = rcp_pool.tile([1, 512], F32, tag="rcp", name=f"rc_{pr}_{qb}_{h2}")
            nc.vector.reciprocal(rcp[:], xsl[HD : HD + 1, :])
            rb = rb_pool.tile([HD, 512], F32, tag="rb", name=f"rb_{pr}_{qb}_{h2}")
            nc.gpsimd.partition_broadcast(rb[:], rcp[0:1, :])
            nc.vector.tensor_tensor(
                xtn_sb[h2 * HD : (h2 + 1) * HD, pr, qb * 512 : (qb + 1) * 512],
                xsl[0:HD, :],
                rb[:],
                op=MULT,
            )

    def sweep(pr, qb):
        xs = x_ps.tile([VW, 1024], F32, tag="xa", name=f"x_{pr}_{qb}")
        for kt in range(KT):
            sweep_iter(pr, qb, kt, xs)
        normalize(pr, qb, xs)

    # ---------------- emission schedule ----------------
    # V first: AV can then drain the exp stream as soon as scores flow.
    # x casts for xv ride on the still-idle ACT engine; xk/xq casts go to
    # the (otherwise idle) GPSIMD engine since ACT is running exps by then.
    wv_bf = load_weight("wv")
    cur = stage_tb(io["xv"], "v", 0, "act")
    for tb in range(TBK):
        nxt = stage_tb(io["xv"], "v", tb + 1, "act") if tb + 1 < TBK else None
        proj_v_tb(wv_bf, cur, tb)
        cur = nxt
        if tb == 0:
            wk_bf = load_weight("wk")

    # K projection, tb-major; first Q block right after K tb0 so the
    # scores/exp stream starts early.  The first pair's attention sweep is
    # interleaved at kt granularity between K-projection ot groups (so the
    # in-order PE queue always has matmuls to run while ACT exps), and
    # exp-ahead bursts for pairs 1-2 keep ACT busy through the rest of the
    # projection phase.
    cur = stage_tb(io["xk"], "k", 0, "pool")
    nxt = stage_tb(io["xk"], "k", 1, "pool")
    proj_qk_tb(wk_bf, cur, 0, kt_sb, bkt)
    wq_bf = load_weight("wq")
    xq0 = stage_tb(io["xq"], "q", 0, "pool")
    proj_qk_tb(wq_bf, xq0, 0, qt_sb, bqt)

    xs0 = x_ps.tile([VW, 1024], F32, tag="xa", name="x_0_0")

    def k_phase_tb(w_bf, xt_t, tb, kts, aheads):
        """One K-proj block with sweep iters between ot pairs."""
        work = [("s", kt) for kt in kts] + [("a", a) for a in aheads]
        for ot in range(NOT):
            proj_qk_ot(w_bf, xt_t, tb, ot, kt_sb, bkt)
            if ot % 2 == 1 and work:
                kind, arg = work.pop(0)
                if kind == "s":
                    sweep_iter(0, 0, arg, xs0)
                else:
                    exp_ahead(arg[0], 0, arg[1])
        for kind, arg in work:
            if kind == "s":
                sweep_iter(0, 0, arg, xs0)
            else:
                exp_ahead(arg[0], 0, arg[1])

    for kt in range(4):
        sweep_iter(0, 0, kt, xs0)
    cur = nxt
    nxt = stage_tb(io["xk"], "k", 2, "pool")
    k_phase_tb(wk_bf, cur, 1, range(4, 8), [(1, range(0, 2))])
    cur = nxt
    nxt = stage_tb(io["xk"], "k", 3, "pool")
    xq1 = stage_tb(io["xq"], "q", 1, "pool")
    k_phase_tb(wk_bf, cur, 2, range(8, 12), [(1, range(2, 4))])
    k_phase_tb(wk_bf, nxt, 3, range(12, 16), [(2, range(0, 2))])
    normalize(0, 0, xs0)
    proj_qk_tb(wq_bf, xq1, 1, qt_sb, bqt)

    # wo load + cast while attention streams
    wo_bf = load_weight("wo")

    # remaining qb0 sweeps
    for pr in range(1, NP):
        sweep(pr, 0)

    # output projection helper (bf16), bias on DVE, fp32 out
    def out_proj(qts, o_ps, ost_pool):
        for qt in qts:
            for ob in range(2):
                ps = o_ps.tile([P, 512], F32, tag="op", name=f"op_{qt}_{ob}")
                for pr in range(NP):
                    nc.tensor.matmul(
                        ps[:],
                        xtn_sb[:, pr, qt * P : (qt + 1) * P],
                        wo_bf[:, pr, ob * 512 : (ob + 1) * 512],
                        start=(pr == 0),
                        stop=(pr == NP - 1),
                    )
                st = ost_pool.tile([P, 512], F32, tag="os", name=f"os_{qt}_{ob}")
                nc.vector.tensor_tensor(
                    st[:], ps[:], bo_bcast[:, ob * 512 : (ob + 1) * 512], op=ADD
                )
                nc.sync.dma_start(
                    out=io["out"][qt * P : (qt + 1) * P, ob * 512 : (ob + 1) * 512],
                    in_=st[:],
                )

    # free the projection-phase pools (LIFO per stack) before phase C pools
    for pool in (proj_ps, wst_pool, xt_pool, xbf_pool, xrow_pool):
        pool.release()

    ost_pool = tc.alloc_tile_pool(name="ost", bufs=3, side="right")
    o_ps = tc.alloc_tile_pool(name="o_ps", bufs=2, space="PSUM")

    # first output half overlaps the qb1 sweeps' exp stream
    out_proj(range(0, SQ // P // 2), o_ps, ost_pool)

    for pr in range(NP):
        sweep(pr, 1)

    out_proj(range(SQ // P // 2, SQ // P), o_ps, ost_pool)

    # final releases, LIFO per stack (left SBUF / right SBUF / PSUM)
    for pool in (
        rcp_pool,
        rb_pool,
        ahead_pool,
        pe_pool,
        w_pool,
        consts,
        persist,
        ost_pool,
        o_ps,
        x_ps,
        s_ps,
    ):
        pool.release()


def build_module(reps=1):
    key = ("nc", reps)
    if key in _CACHE:
        return _CACHE[key]
    nc = bacc.Bacc("TRN2", target_bir_lowering=False, debug=False, num_devices=8)
    io = {}
    io["xq"] = nc.dram_tensor("xq", [SQ, D], F32, kind="ExternalInput").ap()
    io["xk"] = nc.dram_tensor("xk", [SK, D], F32, kind="ExternalInput").ap()
    io["xv"] = nc.dram_tensor("xv", [SK, D], F32, kind="ExternalInput").ap()
    for w in ("wq", "wk", "wv", "wo"):
        io[w] = nc.dram_tensor(w, [D, D], F32, kind="ExternalInput").ap()
    for b in ("bq", "bk", "bv", "bo"):
        io[b] = nc.dram_tensor(b, [D], F32, kind="ExternalInput").ap()
    io["out"] = nc.dram_tensor("out", [SQ, D], F32, kind="ExternalOutput").ap()

    with tile.TileContext(nc) as tc:
        for _ in range(reps):
            _emit(tc, io)
    nc.compile()
    _CACHE[key] = nc
    return nc


LAST_RESULTS = None


def kernel(query, key, value, Wq, bq, Wk, bk, Wv, bv, Wo, bo):
    global LAST_RESULTS
    nc = build_module()
    query = np.ascontiguousarray(np.asarray(query, np.float32))
    key = np.ascontiguousarray(np.asarray(key, np.float32))
    value = np.ascontiguousarray(np.asarray(value, np.float32))
    shared = {
        "wq": np.ascontiguousarray(np.asarray(Wq, np.float32)),
        "wk": np.ascontiguousarray(np.asarray(Wk, np.float32)),
        "wv": np.ascontiguousarray(np.asarray(Wv, np.float32)),
        "wo": np.ascontiguousarray(np.asarray(Wo, np.float32)),
        "bq": np.ascontiguousarray(np.asarray(bq, np.float32)),
        "bk": np.ascontiguousarray(np.asarray(bk, np.float32)),
        "bv": np.ascontiguousarray(np.asarray(bv, np.float32)),
        "bo": np.ascontiguousarray(np.asarray(bo, np.float32)),
    }
    in_maps = []
    for c in range(8):
        b, qh = divmod(c, 2)
        in_maps.append(
            {
                "xq": np.ascontiguousarray(query[b, qh * SQ : (qh + 1) * SQ]),
                "xk": key[b],
                "xv": value[b],
                **shared,
            }
        )
    try:
        res = run_bass_kernel_spmd(nc, in_maps, core_ids=list(range(8)))
    except ModuleNotFoundError:
        # BASS_TRACE was requested but this container lacks the axon NTFF
        # profiling hook module; rerun with tracing disabled.
        os.environ["BASS_NEVER_TRACE"] = "1"
        res = run_bass_kernel_spmd(nc, in_maps, core_ids=list(range(8)))
    LAST_RESULTS = res
    out = np.empty((B, S, D), np.float32)
    for c in range(8):
        b, qh = divmod(c, 2)
        out[b, qh * SQ : (qh + 1) * SQ] = res.results[c]["out"]
    return out


# revision 24
# speedup vs baseline: 1.7299x; 1.7299x over previous
"""Trainium2 Bass kernel for nn_MultiHeadAttention (B=4, S=2048, D=1024, H=16).

Sharding: 8 cores, core c handles batch b=c//2 and query-row half qh=c%2
(1024 query rows), with all 16 heads and the full 2048-key context for
that batch.  No collectives: each core produces a disjoint [1024, 1024]
slab of the output.

v2 dataflow (everything SBUF-resident in bf16 after the PSUM stage):
  - x.T obtained via HWDGE DMA-transpose (bf16, 16x128 xbar tiles), not PE
    transposes: DMA x fp32 -> SBUF, cast fp32->bf16 (ACT early / DVE late),
    dma_start_transpose -> x.T[p, it, tok] = x[tok, it*128+p].
  - Projections in bf16 (weights cast on DVE at load), tb-major.  V first
    (so attention AV can drain as soon as scores flow), then K, then Q.
    Evictions add biases on DVE and write bf16 into resident tiles:
      kt_sb/qt_sb [dim-in-pair(128), pair, token] feature-major,
      vt_sb [key-in-tile(128), kt, head, 65] token-major with a ones column
      (the ones column makes AV also produce the softmax denominator).
  - Attention per (pair, qb): scores via 2 row-packed matmuls (2 heads at
    base partitions 0/64 run concurrently on the PE), exp on ACT with the
    1/sqrt(64) scale folded in (fp32 PSUM -> bf16 SBUF), AV accumulates
    [65, 512] over the 16 key tiles, normalize = DVE reciprocal of row 64 +
    GPSIMD partition-broadcast + DVE multiply -> xtn_sb bf16.
    The exp stream is the single largest engine load (~270us on ACT); the
    emission order interleaves scores/exp bursts into the K-projection
    phase (exp-ahead buffered in the pe pool) so ACT never starves.
  - Output projection in bf16, split by query half so the first half
    overlaps the second half's attention sweeps.
"""

import os
import sys

import numpy as np

sys.path.insert(0, "/opt/trn_rl_repo")

import concourse.bass as bass  # noqa: E402
import concourse.tile as tile  # noqa: E402
from concourse import bacc, mybir  # noqa: E402
from concourse.bass_utils import run_bass_kernel_spmd  # noqa: E402

B, S, D, H = 4, 2048, 1024, 16
HD = D // H          # 64
P = 128
SQ = S // 2          # query rows per core
SK = S               # key rows per core
NIT = D // P         # 8 input-feature tiles
NOT = D // P         # 8 output-feature tiles
KT = SK // P         # 16 key-token tiles
NP = H // 2          # 8 head pairs
VW = HD + 1          # 65: head slice of V plus ones column
TBK = SK // 512      # 4 token blocks (K/V)
TBQ = SQ // 512      # 2 token blocks (Q)

F32 = mybir.dt.float32
BF16 = mybir.dt.bfloat16
EXP = mybir.ActivationFunctionType.Exp
ADD = mybir.AluOpType.add
MULT = mybir.AluOpType.mult

_CACHE: dict = {}


def _emit(tc, io):
    nc = tc.nc

    persist = tc.alloc_tile_pool(name="persist", bufs=1)
    consts = tc.alloc_tile_pool(name="consts", bufs=1)

    # resident bf16 tensors
    kt_sb = persist.tile([P, NP, SK], BF16, tag="ktr")
    qt_sb = persist.tile([P, NP, SQ], BF16, tag="qtr")
    vt_sb = persist.tile([P, KT, H, VW], BF16, tag="vtr")
    xtn_sb = persist.tile([P, NP, SQ], BF16, tag="xtn")

    # biases in per-partition layout: b*[a*128 + p] = tile[p, a]
    bqt = consts.tile([P, NOT], F32, tag="bqt")
    nc.sync.dma_start(out=bqt[:], in_=io["bq"].rearrange("(a p) -> p a", p=P))
    bkt = consts.tile([P, NOT], F32, tag="bkt")
    nc.sync.dma_start(out=bkt[:], in_=io["bk"].rearrange("(a p) -> p a", p=P))
    bv_row = consts.tile([1, D], F32, tag="bvr")
    nc.sync.dma_start(out=bv_row[:], in_=io["bv"].rearrange("(a d) -> a d", a=1))
    bo_row = consts.tile([1, D], F32, tag="bor")
    nc.sync.dma_start(out=bo_row[:], in_=io["bo"].rearrange("(a d) -> a d", a=1))
    bv_bcast = consts.tile([P, D], F32, tag="bvb")
    nc.gpsimd.partition_broadcast(bv_bcast[:], bv_row[0:1, :])
    bo_bcast = consts.tile([P, D], F32, tag="bob")
    nc.gpsimd.partition_broadcast(bo_bcast[:], bo_row[0:1, :])
    # ones column of the augmented V, written once
    nc.vector.memset(vt_sb[:, :, :, HD : HD + 1], 1.0)

    # long-lived pools on the left stack; projection-phase pools on the
    # right stack so they can be released (LIFO) mid-kernel.
    w_pool = tc.alloc_tile_pool(name="w", bufs=2)
    # transient exp tiles (consumed by the next AV) vs exp-ahead tiles
    # (held across the projection phase) live in separate pools so the
    # ahead tiles can't wedge the transient ring.
    pe_pool = tc.alloc_tile_pool(name="pe", bufs=3)
    ahead_pool = tc.alloc_tile_pool(name="ahead", bufs=4)
    rb_pool = tc.alloc_tile_pool(name="rb", bufs=1)
    rcp_pool = tc.alloc_tile_pool(name="rcp", bufs=1)
    xst_pool = tc.alloc_tile_pool(name="xst", bufs=2)
    xrow_pool = tc.alloc_tile_pool(name="xrow", bufs=2, side="right")
    xbf_pool = tc.alloc_tile_pool(name="xbf", bufs=2, side="right")
    xt_pool = tc.alloc_tile_pool(name="xt", bufs=2, side="right")
    wst_pool = tc.alloc_tile_pool(name="wst", bufs=2, side="right")
    # PSUM budget (8 banks): proj 2x[128,512]=2, scores 2x[128,1024]=4,
    # x accumulator 1x[65,1024]=2.  After proj_ps releases, o_ps takes
    # its 2 banks for the output projection that overlaps the qb1 sweeps.
    s_ps = tc.alloc_tile_pool(name="s_ps", bufs=2, space="PSUM")
    x_ps = tc.alloc_tile_pool(name="x_ps", bufs=1, space="PSUM")
    proj_ps = tc.alloc_tile_pool(name="proj_ps", bufs=2, space="PSUM")

    def load_weight(which):
        """DMA fp32 weight row-slices, cast to bf16 on DVE."""
        w_bf = w_pool.tile([P, NIT, D], BF16, tag="w", name=f"w_{which}")
        for it in range(NIT):
            for h in range(2):
                st = wst_pool.tile(
                    [P, 512], F32, tag="wst", name=f"ws_{which}_{it}_{h}"
                )
                nc.sync.dma_start(
                    out=st[:],
                    in_=io[which][it * P : (it + 1) * P, h * 512 : (h + 1) * 512],
                )
                nc.vector.tensor_copy(w_bf[:, it, h * 512 : (h + 1) * 512], st[:])
        return w_bf

    def stage_tb(x_ap, name, tb, cast):
        """One 512-token block: per 128-row tile DMA fp32 -> SBUF, cast to
        bf16, spill to a DRAM bf16 mirror; then one chunked DMA-transpose
        (DRAM [512, 1024] -> SBUF [128, 8, 512]).  The chunked DRAM-path
        transpose measures ~2x faster than per-block SBUF-path transposes
        and keeps the PE entirely out of the transpose business."""
        xd = io[f"{name}_bf"]
        for j in range(4):
            g = tb * 4 + j
            xrow = xrow_pool.tile([P, D], F32, tag="xrow", name=f"xr_{name}_{g}")
            nc.sync.dma_start(out=xrow[:], in_=x_ap[g * P : (g + 1) * P, :])
            xbf = xbf_pool.tile([P, D], BF16, tag="xbf", name=f"xb_{name}_{g}")
            if cast == "act":
                nc.scalar.copy(xbf[:], xrow[:])
            elif cast == "pool":
                nc.gpsimd.tensor_copy(xbf[:], xrow[:])
            else:
                nc.vector.tensor_copy(xbf[:], xrow[:])
            nc.sync.dma_start(out=xd[g * P : (g + 1) * P, :], in_=xbf[:])
        xt_t = xt_pool.tile([P, NIT, 512], BF16, tag="xt", name=f"xt_{name}_{tb}")
        nc.sync.dma_start_transpose(
            out=xt_t[:], in_=xd[tb * 512 : (tb + 1) * 512, :]
        )
        return xt_t

    def proj_qk_ot(w_bf, xt_t, tb, ot, dst, bias):
        ps = proj_ps.tile([P, 512], F32, tag="pj", name=f"p_{tb}_{ot}")
        for it in range(NIT):
            nc.tensor.matmul(
                ps[:],
                w_bf[:, it, ot * P : (ot + 1) * P],
                xt_t[:, it],
                start=(it == 0),
                stop=(it == NIT - 1),
            )
        nc.vector.tensor_scalar_add(
            dst[:, ot, tb * 512 : (tb + 1) * 512], ps[:], bias[:, ot : ot + 1]
        )

    def proj_qk_tb(w_bf, xt_t, tb, dst, bias):
        """Q/K projection for one 512-token block; evict bf16 + bias (DVE)."""
        for ot in range(NOT):
            proj_qk_ot(w_bf, xt_t, tb, ot, dst, bias)

    def proj_v_tb(w_bf, xt_t, tb):
        """V projection for one 512-key block, token-major eviction + bias."""
        for ts in range(4):
            kt = tb * 4 + ts
            for ob in range(2):
                ps = proj_ps.tile([P, 512], F32, tag="pj", name=f"v_{kt}_{ob}")
                for it in range(NIT):
                    nc.tensor.matmul(
                        ps[:],
                        xt_t[:, it, ts * P : (ts + 1) * P],
                        w_bf[:, it, ob * 512 : (ob + 1) * 512],
                        start=(it == 0),
                        stop=(it == NIT - 1),
                    )
                nc.vector.tensor_tensor(
                    vt_sb[:, kt, ob * NOT : (ob + 1) * NOT, 0:HD],
                    ps[:].rearrange("p (h c) -> p h c", c=HD),
                    bv_bcast[:, ob * 512 : (ob + 1) * 512].rearrange(
                        "p (h c) -> p h c", c=HD
                    ),
                    op=ADD,
                )

    pe_buf = {}

    def scores_exp(pr, qb, kt, pool):
        """Row-packed scores for both heads of the pair + exp -> bf16 P."""
        sp = s_ps.tile([P, 1024], F32, tag="sp", name=f"sp_{pr}_{qb}_{kt}")
        for h2 in range(2):
            nc.tensor.matmul(
                sp[:, h2 * 512 : (h2 + 1) * 512],
                kt_sb[h2 * HD : (h2 + 1) * HD, pr, kt * P : (kt + 1) * P],
                qt_sb[h2 * HD : (h2 + 1) * HD, pr, qb * 512 : (qb + 1) * 512],
            )
        pet = pool.tile([P, 1024], BF16, tag="pe", name=f"pe_{pr}_{qb}_{kt}")
        nc.scalar.activation(pet[:], sp[:], EXP, scale=1.0 / 8.0)
        return pet

    def exp_ahead(pr, qb, kts):
        for kt in kts:
            pe_buf[(pr, qb, kt)] = scores_exp(pr, qb, kt, ahead_pool)

    def sweep_iter(pr, qb, kt, xs):
        pet = pe_buf.pop((pr, qb, kt), None)
        if pet is None:
            pet = scores_exp(pr, qb, kt, pe_pool)
        for h2 in range(2):
            nc.tensor.matmul(
                xs[:, h2 * 512 : (h2 + 1) * 512],
                vt_sb[:, kt, 2 * pr + h2, :],
                pet[:, h2 * 512 : (h2 + 1) * 512],
                start=(kt == 0),
                stop=(kt == KT - 1),
            )

    def normalize(pr, qb, xs):
        # Evict the accumulator to SBUF first so the PSUM tile frees for the
        # next sweep's AV immediately; the reciprocal/broadcast/multiply
        # chain then runs off the critical path.
        xst = xst_pool.tile([VW, 1024], F32, tag="xst", name=f"xs_{pr}_{qb}")
        nc.vector.tensor_copy(xst[:], xs[:])
        rcp = rcp_pool.tile([1, 1024], F32, tag="rcp", name=f"rc_{pr}_{qb}")
        nc.vector.reciprocal(rcp[:], xst[HD : HD + 1, :])
        rb = rb_pool.tile([HD, 1024], F32, tag="rb", name=f"rb_{pr}_{qb}")
        nc.gpsimd.partition_broadcast(rb[:], rcp[0:1, :])
        for h2 in range(2):
            nc.vector.tensor_tensor(
                xtn_sb[h2 * HD : (h2 + 1) * HD, pr, qb * 512 : (qb + 1) * 512],
                xst[0:HD, h2 * 512 : (h2 + 1) * 512],
                rb[:, h2 * 512 : (h2 + 1) * 512],
                op=MULT,
            )

    def sweep(pr, qb):
        xs = x_ps.tile([VW, 1024], F32, tag="xa", name=f"x_{pr}_{qb}")
        for kt in range(KT):
            sweep_iter(pr, qb, kt, xs)
        normalize(pr, qb, xs)

    # ---------------- emission schedule ----------------
    # V first: AV can then drain the exp stream as soon as scores flow.
    # x casts for xv ride on the still-idle ACT engine; xk/xq casts go to
    # the (otherwise idle) GPSIMD engine since ACT is running exps by then.
    wv_bf = load_weight("wv")
    cur = stage_tb(io["xv"], "v", 0, "act")
    for tb in range(TBK):
        nxt = stage_tb(io["xv"], "v", tb + 1, "act") if tb + 1 < TBK else None
        proj_v_tb(wv_bf, cur, tb)
        cur = nxt
        if tb == 0:
            wk_bf = load_weight("wk")

    # K projection, tb-major; first Q block right after K tb0 so the
    # scores/exp stream starts early.  The first pair's attention sweep is
    # interleaved at kt granularity between K-projection ot groups (so the
    # in-order PE queue always has matmuls to run while ACT exps), and
    # exp-ahead bursts for pairs 1-2 keep ACT busy through the rest of the
    # projection phase.
    cur = stage_tb(io["xk"], "k", 0, "pool")
    nxt = stage_tb(io["xk"], "k", 1, "pool")
    proj_qk_tb(wk_bf, cur, 0, kt_sb, bkt)
    wq_bf = load_weight("wq")
    xq0 = stage_tb(io["xq"], "q", 0, "pool")
    proj_qk_tb(wq_bf, xq0, 0, qt_sb, bqt)

    xs0 = x_ps.tile([VW, 1024], F32, tag="xa", name="x_0_0")

    def k_phase_tb(w_bf, xt_t, tb, kts, aheads):
        """One K-proj block with sweep iters between ot pairs."""
        work = [("s", kt) for kt in kts] + [("a", a) for a in aheads]
        for ot in range(NOT):
            proj_qk_ot(w_bf, xt_t, tb, ot, kt_sb, bkt)
            if ot % 2 == 1 and work:
                kind, arg = work.pop(0)
                if kind == "s":
                    sweep_iter(0, 0, arg, xs0)
                else:
                    exp_ahead(arg[0], 0, arg[1])
        for kind, arg in work:
            if kind == "s":
                sweep_iter(0, 0, arg, xs0)
            else:
                exp_ahead(arg[0], 0, arg[1])

    for kt in range(4):
        sweep_iter(0, 0, kt, xs0)
    cur = nxt
    nxt = stage_tb(io["xk"], "k", 2, "pool")
    k_phase_tb(wk_bf, cur, 1, range(4, 8), [(1, range(0, 2))])
    cur = nxt
    nxt = stage_tb(io["xk"], "k", 3, "pool")
    xq1 = stage_tb(io["xq"], "q", 1, "pool")
    k_phase_tb(wk_bf, cur, 2, range(8, 12), [(1, range(2, 4))])
    k_phase_tb(wk_bf, nxt, 3, range(12, 16), [])
    normalize(0, 0, xs0)
    proj_qk_tb(wq_bf, xq1, 1, qt_sb, bqt)

    # wo load + cast while attention streams
    wo_bf = load_weight("wo")

    # remaining qb0 sweeps
    for pr in range(1, NP):
        sweep(pr, 0)

    # output projection helper (bf16), bias on DVE, fp32 out
    def out_proj(qts, o_ps, ost_pool):
        for qt in qts:
            for ob in range(2):
                ps = o_ps.tile([P, 512], F32, tag="op", name=f"op_{qt}_{ob}")
                for pr in range(NP):
                    nc.tensor.matmul(
                        ps[:],
                        xtn_sb[:, pr, qt * P : (qt + 1) * P],
                        wo_bf[:, pr, ob * 512 : (ob + 1) * 512],
                        start=(pr == 0),
                        stop=(pr == NP - 1),
                    )
                st = ost_pool.tile([P, 512], F32, tag="os", name=f"os_{qt}_{ob}")
                nc.vector.tensor_tensor(
                    st[:], ps[:], bo_bcast[:, ob * 512 : (ob + 1) * 512], op=ADD
                )
                nc.sync.dma_start(
                    out=io["out"][qt * P : (qt + 1) * P, ob * 512 : (ob + 1) * 512],
                    in_=st[:],
                )

    # free the projection-phase pools (LIFO per stack) before phase C pools
    for pool in (proj_ps, wst_pool, xt_pool, xbf_pool, xrow_pool):
        pool.release()

    ost_pool = tc.alloc_tile_pool(name="ost", bufs=3, side="right")
    o_ps = tc.alloc_tile_pool(name="o_ps", bufs=2, space="PSUM")

    # first output half overlaps the qb1 sweeps' exp stream
    out_proj(range(0, SQ // P // 2), o_ps, ost_pool)

    for pr in range(NP):
        sweep(pr, 1)

    out_proj(range(SQ // P // 2, SQ // P), o_ps, ost_pool)

    # final releases, LIFO per stack (left SBUF / right SBUF / PSUM)
    for pool in (
        xst_pool,
        rcp_pool,
        rb_pool,
        ahead_pool,
        pe_pool,
        w_pool,
        consts,
        persist,
        ost_pool,
        o_ps,
        x_ps,
        s_ps,
    ):
        pool.release()


def build_module(reps=1):
    key = ("nc", reps)
    if key in _CACHE:
        return _CACHE[key]
    nc = bacc.Bacc("TRN2", target_bir_lowering=False, debug=False, num_devices=8)
    io = {}
    io["xq"] = nc.dram_tensor("xq", [SQ, D], F32, kind="ExternalInput").ap()
    io["xk"] = nc.dram_tensor("xk", [SK, D], F32, kind="ExternalInput").ap()
    io["xv"] = nc.dram_tensor("xv", [SK, D], F32, kind="ExternalInput").ap()
    for w in ("wq", "wk", "wv", "wo"):
        io[w] = nc.dram_tensor(w, [D, D], F32, kind="ExternalInput").ap()
    for b in ("bq", "bk", "bv", "bo"):
        io[b] = nc.dram_tensor(b, [D], F32, kind="ExternalInput").ap()
    io["out"] = nc.dram_tensor("out", [SQ, D], F32, kind="ExternalOutput").ap()
    # bf16 mirrors of the inputs, staged for the chunked DMA-transposes
    io["v_bf"] = nc.dram_tensor("v_bf16d", [SK, D], BF16).ap()
    io["k_bf"] = nc.dram_tensor("k_bf16d", [SK, D], BF16).ap()
    io["q_bf"] = nc.dram_tensor("q_bf16d", [SQ, D], BF16).ap()

    with tile.TileContext(nc) as tc:
        for _ in range(reps):
            _emit(tc, io)
    nc.compile()
    _CACHE[key] = nc
    return nc


LAST_RESULTS = None


def kernel(query, key, value, Wq, bq, Wk, bk, Wv, bv, Wo, bo):
    global LAST_RESULTS
    nc = build_module()
    query = np.ascontiguousarray(np.asarray(query, np.float32))
    key = np.ascontiguousarray(np.asarray(key, np.float32))
    value = np.ascontiguousarray(np.asarray(value, np.float32))
    shared = {
        "wq": np.ascontiguousarray(np.asarray(Wq, np.float32)),
        "wk": np.ascontiguousarray(np.asarray(Wk, np.float32)),
        "wv": np.ascontiguousarray(np.asarray(Wv, np.float32)),
        "wo": np.ascontiguousarray(np.asarray(Wo, np.float32)),
        "bq": np.ascontiguousarray(np.asarray(bq, np.float32)),
        "bk": np.ascontiguousarray(np.asarray(bk, np.float32)),
        "bv": np.ascontiguousarray(np.asarray(bv, np.float32)),
        "bo": np.ascontiguousarray(np.asarray(bo, np.float32)),
    }
    in_maps = []
    for c in range(8):
        b, qh = divmod(c, 2)
        in_maps.append(
            {
                "xq": np.ascontiguousarray(query[b, qh * SQ : (qh + 1) * SQ]),
                "xk": key[b],
                "xv": value[b],
                **shared,
            }
        )
    try:
        res = run_bass_kernel_spmd(nc, in_maps, core_ids=list(range(8)))
    except ModuleNotFoundError:
        # BASS_TRACE was requested but this container lacks the axon NTFF
        # profiling hook module; rerun with tracing disabled.
        os.environ["BASS_NEVER_TRACE"] = "1"
        res = run_bass_kernel_spmd(nc, in_maps, core_ids=list(range(8)))
    LAST_RESULTS = res
    out = np.empty((B, S, D), np.float32)
    for c in range(8):
        b, qh = divmod(c, 2)
        out[b, qh * SQ : (qh + 1) * SQ] = res.results[c]["out"]
    return out


# revision 29
# speedup vs baseline: 3.1826x; 1.8397x over previous
"""Trainium2 Bass kernel for nn_MultiHeadAttention (B=4, S=2048, D=1024, H=16).

Sharding: 8 cores, core c handles batch b=c//2 and query-row half qh=c%2
(1024 query rows), with all 16 heads and the full 2048-key context for
that batch.  No collectives are needed: each core produces a disjoint
[1024, 1024] slab of the output.

Per-core dataflow (all matmuls in fp32r = full PE rate at FP22 precision):
  Phase A: transpose inputs via PE, project Q/K/V.
           - Q.T, K.T produced feature-major ([d, token]); Q.T spilled to a
             DRAM scratch, K.T resident in SBUF.
           - V produced token-major with a ones column appended per head
             (65-wide head stride) and spilled to DRAM scratch.
  Phase B: per head-pair p (K.T rows are 2 heads x 64 dims = 128 partitions):
           scores.T[k, q] via row-packed matmuls (2 heads concurrently on the
           PE via tile_position row groups), exp on the Scalar engine with the
           1/sqrt(64) folded into the activation scale, then x_aug = V_aug.T @ P
           which yields x.T rows 0..63 and the softmax denominator in row 64.
           Normalization: reciprocal of row 64 on the Vector engine, GPSIMD
           partition-broadcast across the 64 head dims, multiply on DVE.
  Phase C: output projection out = x.T.T @ Wo + bo, accumulated over all 8
           head-pair row blocks, written back token-major.
"""

import os
import sys

import numpy as np

sys.path.insert(0, "/opt/trn_rl_repo")

import concourse.bass as bass  # noqa: E402
import concourse.tile as tile  # noqa: E402
from concourse import bacc, mybir  # noqa: E402
from concourse.bass_utils import run_bass_kernel_spmd  # noqa: E402
from concourse.masks import make_identity  # noqa: E402

B, S, D, H = 4, 2048, 1024, 16
HD = D // H          # 64
P = 128
SQ = S // 2          # query rows per core
SK = S               # key rows per core
NIT = D // P         # 8 input-feature tiles
NOT = D // P         # 8 output-feature tiles
KT = SK // P         # 16 key-token tiles
NQB = SQ // 512      # 2 query blocks of 512
NP = H // 2          # 8 head pairs
VW = HD + 1          # 65: head slice of V plus ones column

F32 = mybir.dt.float32
F32R = mybir.dt.float32r
EXP = mybir.ActivationFunctionType.Exp

_CACHE: dict = {}


def _r(ap):
    """fp32r view of an fp32 AP (full-rate PE matmul, FP22 mantissa)."""
    return ap.bitcast(F32R)


def _emit(tc, io):
    nc = tc.nc

    with (
        tc.tile_pool(name="persist", bufs=1) as persist,
        tc.tile_pool(name="consts", bufs=1) as consts,
    ):
        # K.T resident: [dim-in-pair(128), pair, key-token]
        kt_sb = persist.tile([P, NP, SK], F32, tag="ktr")

        # The BIR verifier requires every producer of an fp32r matmul operand
        # to write with an fp32r-typed output, hence the copies below and the
        # _r() on DMA/compute outputs throughout.
        ident_f32 = consts.tile([P, P], F32, tag="identf")
        make_identity(nc, ident_f32)
        ident = consts.tile([P, P], F32, tag="ident")
        nc.vector.tensor_copy(_r(ident[:]), ident_f32[:])
        # biases in per-partition layout: b*[ot*128 + p] = tile[p, ot]
        bqt = consts.tile([P, NOT], F32, tag="bqt")
        nc.sync.dma_start(out=bqt[:], in_=io["bq"].rearrange("(a p) -> p a", p=P))
        bkt = consts.tile([P, NOT], F32, tag="bkt")
        nc.sync.dma_start(out=bkt[:], in_=io["bk"].rearrange("(a p) -> p a", p=P))
        bv_row = consts.tile([1, D], F32, tag="bvr")
        nc.sync.dma_start(out=bv_row[:], in_=io["bv"].rearrange("(a d) -> a d", a=1))
        bo_row = consts.tile([1, D], F32, tag="bor")
        nc.sync.dma_start(out=bo_row[:], in_=io["bo"].rearrange("(a d) -> a d", a=1))
        # biases broadcast to all 128 partitions once (GPSIMD) so projection
        # evictions can add them with a tensor_tensor instead of K=1 matmuls
        bv_bcast = consts.tile([P, D], F32, tag="bvb")
        nc.gpsimd.partition_broadcast(bv_bcast[:], bv_row[0:1, :])
        bo_bcast = consts.tile([P, D], F32, tag="bob")
        nc.gpsimd.partition_broadcast(bo_bcast[:], bo_row[0:1, :])

        # ---------------- Phase A: Q/K/V projections ----------------
        with (
            tc.tile_pool(name="wbuf", bufs=2) as wpool,
            tc.tile_pool(name="xrow", bufs=4) as xrow_pool,
            tc.tile_pool(name="xtblk", bufs=2) as xt_pool,
            tc.tile_pool(name="astage", bufs=3) as stage_pool,
            tc.tile_pool(name="tp_ps", bufs=5, space="PSUM") as tp_psum,
            tc.tile_pool(name="proj_ps", bufs=3, space="PSUM") as proj_psum,
        ):

            def load_w(which):
                w_sb = wpool.tile([P, NIT, D], F32, tag="w", name=f"w_{which}")
                for it in range(NIT):
                    nc.sync.dma_start(
                        out=_r(w_sb[:, it]), in_=_r(io[which][it * P : (it + 1) * P, :])
                    )
                return w_sb

            def transpose_block(x_ap, t0, nt, dst):
                """dst[:, it, ts*128:...] = x_ap[t0:t0+nt*128, :].T via PE."""
                for ts in range(nt):
                    xrow = xrow_pool.tile([P, D], F32, tag="xrow", name=f"xr_{t0}_{ts}")
                    nc.sync.dma_start(
                        out=_r(xrow[:]),
                        in_=_r(x_ap[t0 + ts * P : t0 + (ts + 1) * P, :]),
                    )
                    for it in range(NIT):
                        tp = tp_psum.tile([P, P], F32, tag="tp", name=f"tp_{ts}_{it}")
                        nc.tensor.transpose(
                            _r(tp[:]), _r(xrow[:, it * P : (it + 1) * P]), _r(ident[:])
                        )
                        # drain PSUM on both DVE and the (phase-A idle) ACT
                        # engine so the copy drain stops gating each block
                        if it % 2 == 0:
                            nc.vector.tensor_copy(
                                _r(dst[:, it, ts * P : (ts + 1) * P]), tp[:]
                            )
                        else:
                            nc.scalar.copy(
                                _r(dst[:, it, ts * P : (ts + 1) * P]), tp[:]
                            )

            # --- Q projection -> qt_dram [o, t] (transposed) ---
            # transpose loads emitted before the weight DMA so the PE's first
            # work isn't queued behind a 4MB weight transfer
            xtq = []
            for tb in range(SQ // 512):
                blk = xt_pool.tile([P, NIT, 512], F32, tag="xt", name=f"xtq_{tb}")
                transpose_block(io["xq"], tb * 512, 4, blk)
                xtq.append(blk)
                if tb == 0:
                    w_sb = load_w("wq")
            for tb in range(SQ // 512):
                xt_blk = xtq[tb]
                for ot in range(NOT):
                    ps = proj_psum.tile([P, 512], F32, tag="pj", name=f"qp_{tb}_{ot}")
                    for it in range(NIT):
                        nc.tensor.matmul(
                            ps[:],
                            _r(w_sb[:, it, ot * P : (ot + 1) * P]),
                            _r(xt_blk[:, it]),
                            start=(it == 0),
                            stop=(it == NIT - 1),
                        )
                    st = stage_pool.tile([P, 512], F32, tag="qs", name=f"qs_{tb}_{ot}")
                    nc.scalar.add(st[:], ps[:], bqt[:, ot : ot + 1])
                    nc.sync.dma_start(
                        out=io["qt_dram"][
                            ot * P : (ot + 1) * P, tb * 512 : (tb + 1) * 512
                        ],
                        in_=st[:],
                    )

            # --- K projection -> kt_sb resident (transposed) ---
            w_sb = load_w("wk")
            cur_k = xt_pool.tile([P, NIT, 512], F32, tag="xt", name="xtk_0")
            transpose_block(io["xk"], 0, 4, cur_k)
            for tb in range(SK // 512):
                xt_blk = cur_k
                if tb + 1 < SK // 512:
                    cur_k = xt_pool.tile(
                        [P, NIT, 512], F32, tag="xt", name=f"xtk_{tb + 1}"
                    )
                    transpose_block(io["xk"], (tb + 1) * 512, 4, cur_k)
                for ot in range(NOT):
                    ps = proj_psum.tile([P, 512], F32, tag="pj", name=f"kp_{tb}_{ot}")
                    for it in range(NIT):
                        nc.tensor.matmul(
                            ps[:],
                            _r(w_sb[:, it, ot * P : (ot + 1) * P]),
                            _r(xt_blk[:, it]),
                            start=(it == 0),
                            stop=(it == NIT - 1),
                        )
                    nc.scalar.add(
                        _r(kt_sb[:, ot, tb * 512 : (tb + 1) * 512]),
                        ps[:],
                        bkt[:, ot : ot + 1],
                    )

            # --- V projection -> v_dram token-major, 65-stride per head ---
            w_sb = load_w("wv")
            cur_v = xt_pool.tile([P, NIT, 512], F32, tag="xt", name="xtv_0")
            transpose_block(io["xv"], 0, 4, cur_v)
            for tb in range(SK // 512):
                xt_blk = cur_v
                if tb + 1 < SK // 512:
                    cur_v = xt_pool.tile(
                        [P, NIT, 512], F32, tag="xt", name=f"xtv_{tb + 1}"
                    )
                    transpose_block(io["xv"], (tb + 1) * 512, 4, cur_v)
                for ts in range(4):
                    kt = tb * 4 + ts
                    for ob in range(2):
                        ps = proj_psum.tile(
                            [P, 512], F32, tag="pj", name=f"vp_{kt}_{ob}"
                        )
                        for it in range(NIT):
                            nc.tensor.matmul(
                                ps[:],
                                _r(xt_blk[:, it, ts * P : (ts + 1) * P]),
                                _r(w_sb[:, it, ob * 512 : (ob + 1) * 512]),
                                start=(it == 0),
                                stop=(it == NIT - 1),
                            )
                        vst = stage_pool.tile(
                            [P, 8, VW], F32, tag="vs", name=f"vs_{kt}_{ob}"
                        )
                        nc.vector.tensor_tensor(
                            _r(vst[:, :, 0:HD]),
                            ps[:].rearrange("p (h c) -> p h c", c=HD),
                            bv_bcast[:, ob * 512 : (ob + 1) * 512].rearrange(
                                "p (h c) -> p h c", c=HD
                            ),
                            op=mybir.AluOpType.add,
                        )
                        nc.vector.memset(vst[:, :, HD : HD + 1], 1.0)
                        nc.sync.dma_start(
                            out=_r(
                                io["v_dram"][
                                    kt * P : (kt + 1) * P,
                                    ob * 8 * VW : (ob + 1) * 8 * VW,
                                ]
                            ),
                            in_=_r(vst[:]),
                        )

        # ---------------- Phase B: attention per head pair ----------------
        v_view = io["v_dram"].rearrange("(k p) (g c) -> p k g c", p=P, c=VW)
        with (
            tc.tile_pool(name="bc", bufs=1) as bc_pool,
            tc.tile_pool(name="qtp", bufs=2) as qtp_pool,
            tc.tile_pool(name="vpair", bufs=2) as vp_pool,
            tc.tile_pool(name="pexp", bufs=4) as p_pool,
            tc.tile_pool(name="rcp", bufs=2) as rcp_pool,
            tc.tile_pool(name="rbs", bufs=2) as rbs_pool,
        ):
            # normalized x.T resident: [dim-in-pair(128), pair, query-token]
            xtn_sb = bc_pool.tile([P, NP, SQ], F32, tag="xtn")
            # prefetch Wo during phase B so phase C doesn't stall on it
            wo_sb = bc_pool.tile([P, NP, D], F32, tag="wo")
            for it in range(NP):
                nc.sync.dma_start(
                    out=_r(wo_sb[:, it]), in_=_r(io["wo"][it * P : (it + 1) * P, :])
                )
            with (
                tc.tile_pool(name="s_ps", bufs=3, space="PSUM") as s_psum,
                tc.tile_pool(name="x_ps", bufs=2, space="PSUM") as x_psum,
            ):
                for pr in range(NP):
                    qtp = qtp_pool.tile([P, SQ], F32, tag="qtp", name=f"qtp_{pr}")
                    nc.sync.dma_start(
                        out=_r(qtp[:]), in_=_r(io["qt_dram"][pr * P : (pr + 1) * P, :])
                    )
                    vp = vp_pool.tile([P, KT, 2, VW], F32, tag="vp", name=f"vp_{pr}")
                    nc.sync.dma_start(
                        out=_r(vp[:]), in_=_r(v_view[:, :, 2 * pr : 2 * pr + 2, :])
                    )
                    for qb in range(NQB):
                        xs = [
                            x_psum.tile(
                                [VW, 512], F32, tag="xa", name=f"x_{pr}_{qb}_{h2}"
                            )
                            for h2 in range(2)
                        ]
                        for kt in range(KT):
                            sp = s_psum.tile(
                                [P, 1024], F32, tag="sp", name=f"sp_{pr}_{qb}_{kt}"
                            )
                            for h2 in range(2):
                                nc.tensor.matmul(
                                    sp[:, h2 * 512 : (h2 + 1) * 512],
                                    _r(
                                        kt_sb[
                                            h2 * HD : (h2 + 1) * HD,
                                            pr,
                                            kt * P : (kt + 1) * P,
                                        ]
                                    ),
                                    _r(
                                        qtp[
                                            h2 * HD : (h2 + 1) * HD,
                                            qb * 512 : (qb + 1) * 512,
                                        ]
                                    ),
                                )
                            pe = p_pool.tile(
                                [P, 1024], F32, tag="pe", name=f"pe_{pr}_{qb}_{kt}"
                            )
                            nc.scalar.activation(_r(pe[:]), sp[:], EXP, scale=1.0 / 8.0)
                            for h2 in range(2):
                                nc.tensor.matmul(
                                    xs[h2][:],
                                    _r(vp[:, kt, h2]),
                                    _r(pe[:, h2 * 512 : (h2 + 1) * 512]),
                                    start=(kt == 0),
                                    stop=(kt == KT - 1),
                                )
                        for h2 in range(2):
                            # evict x_aug to SBUF right away so the PSUM bank
                            # frees for the next q-block's AV accumulation
                            xst = rbs_pool.tile(
                                [VW, 512], F32, tag="xst", name=f"xe_{pr}_{qb}_{h2}"
                            )
                            nc.vector.tensor_copy(xst[:], xs[h2][:])
                            rcp = rcp_pool.tile(
                                [1, 512], F32, tag="rcp", name=f"rc_{pr}_{qb}_{h2}"
                            )
                            nc.vector.reciprocal(rcp[:], xst[HD : HD + 1, :])
                            # denominator broadcast across the 64 head dims on
                            # the otherwise-idle GPSIMD engine
                            rb = rbs_pool.tile(
                                [HD, 512], F32, tag="rb", name=f"rb_{pr}_{qb}_{h2}"
                            )
                            nc.gpsimd.partition_broadcast(rb[:], rcp[0:1, :])
                            nc.vector.tensor_tensor(
                                _r(
                                    xtn_sb[
                                        h2 * HD : (h2 + 1) * HD,
                                        pr,
                                        qb * 512 : (qb + 1) * 512,
                                    ]
                                ),
                                xst[0:HD, :],
                                rb[:],
                                op=mybir.AluOpType.mult,
                            )

            # ---------------- Phase C: output projection ----------------
            with (
                tc.tile_pool(name="ostage", bufs=4) as ost_pool,
                tc.tile_pool(name="o_ps", bufs=6, space="PSUM") as o_psum,
            ):
                for qt in range(SQ // P):
                    for ob in range(2):
                        ps = o_psum.tile(
                            [P, 512], F32, tag="op", name=f"op_{qt}_{ob}"
                        )
                        for pr in range(NP):
                            nc.tensor.matmul(
                                ps[:],
                                _r(xtn_sb[:, pr, qt * P : (qt + 1) * P]),
                                _r(wo_sb[:, pr, ob * 512 : (ob + 1) * 512]),
                                start=(pr == 0),
                                stop=(pr == NP - 1),
                            )
                        st = ost_pool.tile(
                            [P, 512], F32, tag="os", name=f"os_{qt}_{ob}"
                        )
                        nc.vector.tensor_tensor(
                            st[:],
                            ps[:],
                            bo_bcast[:, ob * 512 : (ob + 1) * 512],
                            op=mybir.AluOpType.add,
                        )
                        nc.sync.dma_start(
                            out=io["out"][
                                qt * P : (qt + 1) * P, ob * 512 : (ob + 1) * 512
                            ],
                            in_=st[:],
                        )




def build_module(reps=1):
    key = ("nc", reps)
    if key in _CACHE:
        return _CACHE[key]
    nc = bacc.Bacc("TRN2", target_bir_lowering=False, debug=False, num_devices=8)
    io = {}
    io["xq"] = nc.dram_tensor("xq", [SQ, D], F32, kind="ExternalInput").ap()
    io["xk"] = nc.dram_tensor("xk", [SK, D], F32, kind="ExternalInput").ap()
    io["xv"] = nc.dram_tensor("xv", [SK, D], F32, kind="ExternalInput").ap()
    for w in ("wq", "wk", "wv", "wo"):
        io[w] = nc.dram_tensor(w, [D, D], F32, kind="ExternalInput").ap()
    for b in ("bq", "bk", "bv", "bo"):
        io[b] = nc.dram_tensor(b, [D], F32, kind="ExternalInput").ap()
    io["out"] = nc.dram_tensor("out", [SQ, D], F32, kind="ExternalOutput").ap()
    io["qt_dram"] = nc.dram_tensor("qt_scratch", [D, SQ], F32).ap()
    io["v_dram"] = nc.dram_tensor("v_scratch", [SK, H * VW], F32).ap()

    with tile.TileContext(nc) as tc:
        for _ in range(reps):
            _emit(tc, io)
    nc.compile()
    _CACHE[key] = nc
    return nc


LAST_RESULTS = None


def kernel(query, key, value, Wq, bq, Wk, bk, Wv, bv, Wo, bo):
    global LAST_RESULTS
    nc = build_module()
    query = np.ascontiguousarray(np.asarray(query, np.float32))
    key = np.ascontiguousarray(np.asarray(key, np.float32))
    value = np.ascontiguousarray(np.asarray(value, np.float32))
    shared = {
        "wq": np.ascontiguousarray(np.asarray(Wq, np.float32)),
        "wk": np.ascontiguousarray(np.asarray(Wk, np.float32)),
        "wv": np.ascontiguousarray(np.asarray(Wv, np.float32)),
        "wo": np.ascontiguousarray(np.asarray(Wo, np.float32)),
        "bq": np.ascontiguousarray(np.asarray(bq, np.float32)),
        "bk": np.ascontiguousarray(np.asarray(bk, np.float32)),
        "bv": np.ascontiguousarray(np.asarray(bv, np.float32)),
        "bo": np.ascontiguousarray(np.asarray(bo, np.float32)),
    }
    in_maps = []
    for c in range(8):
        b, qh = divmod(c, 2)
        in_maps.append(
            {
                "xq": np.ascontiguousarray(query[b, qh * SQ : (qh + 1) * SQ]),
                "xk": key[b],
                "xv": value[b],
                **shared,
            }
        )
    try:
        res = run_bass_kernel_spmd(nc, in_maps, core_ids=list(range(8)))
    except ModuleNotFoundError:
        # BASS_TRACE was requested but this container lacks the axon NTFF
        # profiling hook module; rerun with tracing disabled.
        os.environ["BASS_NEVER_TRACE"] = "1"
        res = run_bass_kernel_spmd(nc, in_maps, core_ids=list(range(8)))
    LAST_RESULTS = res
    out = np.empty((B, S, D), np.float32)
    for c in range(8):
        b, qh = divmod(c, 2)
        out[b, qh * SQ : (qh + 1) * SQ] = res.results[c]["out"]
    return out



# revision 31
# speedup vs baseline: 3.2440x; 1.0193x over previous
"""Trainium2 Bass kernel for nn_MultiHeadAttention (B=4, S=2048, D=1024, H=16).

Sharding: 8 cores, core c handles batch b=c//2 and query-row half qh=c%2
(1024 query rows), with all 16 heads and the full 2048-key context for
that batch.  No collectives are needed: each core produces a disjoint
[1024, 1024] slab of the output.

Per-core dataflow (all matmuls in fp32r = full PE rate at FP22 precision):
  Phase A: transpose inputs via PE, project Q/K/V.
           - Q.T, K.T produced feature-major ([d, token]); Q.T spilled to a
             DRAM scratch, K.T resident in SBUF.
           - V produced token-major with a ones column appended per head
             (65-wide head stride) and spilled to DRAM scratch.
  Phase B: per head-pair p (K.T rows are 2 heads x 64 dims = 128 partitions):
           scores.T[k, q] via row-packed matmuls (2 heads concurrently on the
           PE via tile_position row groups), exp on the Scalar engine with the
           1/sqrt(64) folded into the activation scale, then x_aug = V_aug.T @ P
           which yields x.T rows 0..63 and the softmax denominator in row 64.
           Normalization: reciprocal of row 64 on the Vector engine, GPSIMD
           partition-broadcast across the 64 head dims, multiply on DVE.
  Phase C: output projection out = x.T.T @ Wo + bo, accumulated over all 8
           head-pair row blocks, written back token-major.
"""

import os
import sys

import numpy as np

sys.path.insert(0, "/opt/trn_rl_repo")

import concourse.bass as bass  # noqa: E402
import concourse.tile as tile  # noqa: E402
from concourse import bacc, mybir  # noqa: E402
from concourse.bass_utils import run_bass_kernel_spmd  # noqa: E402
from concourse.masks import make_identity  # noqa: E402

B, S, D, H = 4, 2048, 1024, 16
HD = D // H          # 64
P = 128
SQ = S // 2          # query rows per core
SK = S               # key rows per core
NIT = D // P         # 8 input-feature tiles
NOT = D // P         # 8 output-feature tiles
KT = SK // P         # 16 key-token tiles
NQB = SQ // 512      # 2 query blocks of 512
NP = H // 2          # 8 head pairs
VW = HD + 1          # 65: head slice of V plus ones column

F32 = mybir.dt.float32
F32R = mybir.dt.float32r
BF16 = mybir.dt.bfloat16
EXP = mybir.ActivationFunctionType.Exp

_CACHE: dict = {}


def _r(ap):
    """fp32r view of an fp32 AP (full-rate PE matmul, FP22 mantissa)."""
    return ap.bitcast(F32R)


def _emit(tc, io):
    nc = tc.nc

    with (
        tc.tile_pool(name="persist", bufs=1) as persist,
        tc.tile_pool(name="consts", bufs=1) as consts,
    ):
        # K.T resident: [dim-in-pair(128), pair, key-token]
        kt_sb = persist.tile([P, NP, SK], BF16, tag="ktr")

        # The BIR verifier requires every producer of an fp32r matmul operand
        # to write with an fp32r-typed output, hence the copies below and the
        # _r() on DMA/compute outputs throughout.
        ident_f32 = consts.tile([P, P], F32, tag="identf")
        make_identity(nc, ident_f32)
        ident = consts.tile([P, P], F32, tag="ident")
        nc.vector.tensor_copy(_r(ident[:]), ident_f32[:])
        # biases in per-partition layout: b*[ot*128 + p] = tile[p, ot]
        bqt = consts.tile([P, NOT], F32, tag="bqt")
        nc.sync.dma_start(out=bqt[:], in_=io["bq"].rearrange("(a p) -> p a", p=P))
        bkt = consts.tile([P, NOT], F32, tag="bkt")
        nc.sync.dma_start(out=bkt[:], in_=io["bk"].rearrange("(a p) -> p a", p=P))
        bv_row = consts.tile([1, D], F32, tag="bvr")
        nc.sync.dma_start(out=bv_row[:], in_=io["bv"].rearrange("(a d) -> a d", a=1))
        bo_row = consts.tile([1, D], F32, tag="bor")
        nc.sync.dma_start(out=bo_row[:], in_=io["bo"].rearrange("(a d) -> a d", a=1))
        # biases broadcast to all 128 partitions once (GPSIMD) so projection
        # evictions can add them with a tensor_tensor instead of K=1 matmuls
        bv_bcast = consts.tile([P, D], F32, tag="bvb")
        nc.gpsimd.partition_broadcast(bv_bcast[:], bv_row[0:1, :])
        bo_bcast = consts.tile([P, D], F32, tag="bob")
        nc.gpsimd.partition_broadcast(bo_bcast[:], bo_row[0:1, :])

        # ---------------- Phase A: Q/K/V projections ----------------
        with (
            tc.tile_pool(name="wbuf", bufs=2) as wpool,
            tc.tile_pool(name="xrow", bufs=6) as xrow_pool,
            tc.tile_pool(name="xtblk", bufs=2) as xt_pool,
            tc.tile_pool(name="astage", bufs=5) as stage_pool,
            tc.tile_pool(name="tp_ps", bufs=5, space="PSUM") as tp_psum,
            tc.tile_pool(name="proj_ps", bufs=3, space="PSUM") as proj_psum,
        ):

            def load_w(which):
                w_sb = wpool.tile([P, NIT, D], F32, tag="w", name=f"w_{which}")
                for it in range(NIT):
                    nc.sync.dma_start(
                        out=_r(w_sb[:, it]), in_=_r(io[which][it * P : (it + 1) * P, :])
                    )
                return w_sb

            def transpose_block(x_ap, t0, nt, dst):
                """dst[:, it, ts*128:...] = x_ap[t0:t0+nt*128, :].T via PE."""
                for ts in range(nt):
                    xrow = xrow_pool.tile([P, D], F32, tag="xrow", name=f"xr_{t0}_{ts}")
                    nc.sync.dma_start(
                        out=_r(xrow[:]),
                        in_=_r(x_ap[t0 + ts * P : t0 + (ts + 1) * P, :]),
                    )
                    for it in range(NIT):
                        tp = tp_psum.tile([P, P], F32, tag="tp", name=f"tp_{ts}_{it}")
                        nc.tensor.transpose(
                            _r(tp[:]), _r(xrow[:, it * P : (it + 1) * P]), _r(ident[:])
                        )
                        # drain PSUM on both DVE and the (phase-A idle) ACT
                        # engine so the copy drain stops gating each block
                        if it % 2 == 0:
                            nc.vector.tensor_copy(
                                _r(dst[:, it, ts * P : (ts + 1) * P]), tp[:]
                            )
                        else:
                            nc.scalar.copy(
                                _r(dst[:, it, ts * P : (ts + 1) * P]), tp[:]
                            )

            # --- Q projection -> qt_dram [o, t] (transposed) ---
            # transpose loads emitted before the weight DMA so the PE's first
            # work isn't queued behind a 4MB weight transfer
            xtq = []
            for tb in range(SQ // 512):
                blk = xt_pool.tile([P, NIT, 512], F32, tag="xt", name=f"xtq_{tb}")
                transpose_block(io["xq"], tb * 512, 4, blk)
                xtq.append(blk)
                if tb == 0:
                    w_sb = load_w("wq")
            for tb in range(SQ // 512):
                xt_blk = xtq[tb]
                for ot in range(NOT):
                    ps = proj_psum.tile([P, 512], F32, tag="pj", name=f"qp_{tb}_{ot}")
                    for it in range(NIT):
                        nc.tensor.matmul(
                            ps[:],
                            _r(w_sb[:, it, ot * P : (ot + 1) * P]),
                            _r(xt_blk[:, it]),
                            start=(it == 0),
                            stop=(it == NIT - 1),
                        )
                    st = stage_pool.tile([P, 512], BF16, tag="qs", name=f"qs_{tb}_{ot}")
                    nc.scalar.add(st[:], ps[:], bqt[:, ot : ot + 1])
                    nc.sync.dma_start(
                        out=io["qt_dram"][
                            ot * P : (ot + 1) * P, tb * 512 : (tb + 1) * 512
                        ],
                        in_=st[:],
                    )

            # --- K projection -> kt_sb resident (transposed) ---
            w_sb = load_w("wk")
            cur_k = xt_pool.tile([P, NIT, 512], F32, tag="xt", name="xtk_0")
            transpose_block(io["xk"], 0, 4, cur_k)
            for tb in range(SK // 512):
                xt_blk = cur_k
                if tb + 1 < SK // 512:
                    cur_k = xt_pool.tile(
                        [P, NIT, 512], F32, tag="xt", name=f"xtk_{tb + 1}"
                    )
                    transpose_block(io["xk"], (tb + 1) * 512, 4, cur_k)
                for ot in range(NOT):
                    ps = proj_psum.tile([P, 512], F32, tag="pj", name=f"kp_{tb}_{ot}")
                    for it in range(NIT):
                        nc.tensor.matmul(
                            ps[:],
                            _r(w_sb[:, it, ot * P : (ot + 1) * P]),
                            _r(xt_blk[:, it]),
                            start=(it == 0),
                            stop=(it == NIT - 1),
                        )
                    nc.scalar.add(
                        kt_sb[:, ot, tb * 512 : (tb + 1) * 512],
                        ps[:],
                        bkt[:, ot : ot + 1],
                    )

            # --- V projection -> v_dram token-major, 65-stride per head ---
            w_sb = load_w("wv")
            cur_v = xt_pool.tile([P, NIT, 512], F32, tag="xt", name="xtv_0")
            transpose_block(io["xv"], 0, 4, cur_v)
            for tb in range(SK // 512):
                xt_blk = cur_v
                if tb + 1 < SK // 512:
                    cur_v = xt_pool.tile(
                        [P, NIT, 512], F32, tag="xt", name=f"xtv_{tb + 1}"
                    )
                    transpose_block(io["xv"], (tb + 1) * 512, 4, cur_v)
                for ts in range(4):
                    kt = tb * 4 + ts
                    for ob in range(2):
                        ps = proj_psum.tile(
                            [P, 512], F32, tag="pj", name=f"vp_{kt}_{ob}"
                        )
                        for it in range(NIT):
                            nc.tensor.matmul(
                                ps[:],
                                _r(xt_blk[:, it, ts * P : (ts + 1) * P]),
                                _r(w_sb[:, it, ob * 512 : (ob + 1) * 512]),
                                start=(it == 0),
                                stop=(it == NIT - 1),
                            )
                        vst = stage_pool.tile(
                            [P, 8, VW], BF16, tag="vs", name=f"vs_{kt}_{ob}"
                        )
                        nc.vector.tensor_tensor(
                            vst[:, :, 0:HD],
                            ps[:].rearrange("p (h c) -> p h c", c=HD),
                            bv_bcast[:, ob * 512 : (ob + 1) * 512].rearrange(
                                "p (h c) -> p h c", c=HD
                            ),
                            op=mybir.AluOpType.add,
                        )
                        nc.vector.memset(vst[:, :, HD : HD + 1], 1.0)
                        nc.sync.dma_start(
                            out=io["v_dram"][
                                kt * P : (kt + 1) * P,
                                ob * 8 * VW : (ob + 1) * 8 * VW,
                            ],
                            in_=vst[:],
                        )

        # ---------------- Phase B: attention per head pair ----------------
        v_view = io["v_dram"].rearrange("(k p) (g c) -> p k g c", p=P, c=VW)
        with (
            tc.tile_pool(name="bc", bufs=1) as bc_pool,
            tc.tile_pool(name="qtp", bufs=3) as qtp_pool,
            tc.tile_pool(name="vpair", bufs=3) as vp_pool,
            tc.tile_pool(name="pexp", bufs=12) as p_pool,
            tc.tile_pool(name="rcp", bufs=3) as rcp_pool,
            tc.tile_pool(name="rbs", bufs=3) as rbs_pool,
        ):
            # normalized x.T resident: [dim-in-pair(128), pair, query-token]
            xtn_sb = bc_pool.tile([P, NP, SQ], F32, tag="xtn")
            # prefetch Wo during phase B so phase C doesn't stall on it
            wo_sb = bc_pool.tile([P, NP, D], F32, tag="wo")
            for it in range(NP):
                nc.sync.dma_start(
                    out=_r(wo_sb[:, it]), in_=_r(io["wo"][it * P : (it + 1) * P, :])
                )
            with (
                tc.tile_pool(name="s_ps", bufs=3, space="PSUM") as s_psum,
                tc.tile_pool(name="x_ps", bufs=2, space="PSUM") as x_psum,
            ):
                for pr in range(NP):
                    qtp = qtp_pool.tile([P, SQ], BF16, tag="qtp", name=f"qtp_{pr}")
                    nc.sync.dma_start(
                        out=qtp[:], in_=io["qt_dram"][pr * P : (pr + 1) * P, :]
                    )
                    vp = vp_pool.tile([P, KT, 2, VW], BF16, tag="vp", name=f"vp_{pr}")
                    nc.sync.dma_start(
                        out=vp[:], in_=v_view[:, :, 2 * pr : 2 * pr + 2, :]
                    )
                    for qb in range(NQB):
                        xs = [
                            x_psum.tile(
                                [VW, 512], F32, tag="xa", name=f"x_{pr}_{qb}_{h2}"
                            )
                            for h2 in range(2)
                        ]
                        for kt in range(KT):
                            sp = s_psum.tile(
                                [P, 1024], F32, tag="sp", name=f"sp_{pr}_{qb}_{kt}"
                            )
                            for h2 in range(2):
                                nc.tensor.matmul(
                                    sp[:, h2 * 512 : (h2 + 1) * 512],
                                    kt_sb[
                                        h2 * HD : (h2 + 1) * HD,
                                        pr,
                                        kt * P : (kt + 1) * P,
                                    ],
                                    qtp[
                                        h2 * HD : (h2 + 1) * HD,
                                        qb * 512 : (qb + 1) * 512,
                                    ],
                                )
                            pe = p_pool.tile(
                                [P, 1024], BF16, tag="pe", name=f"pe_{pr}_{qb}_{kt}"
                            )
                            nc.scalar.activation(pe[:], sp[:], EXP, scale=1.0 / 8.0)
                            for h2 in range(2):
                                nc.tensor.matmul(
                                    xs[h2][:],
                                    vp[:, kt, h2],
                                    pe[:, h2 * 512 : (h2 + 1) * 512],
                                    start=(kt == 0),
                                    stop=(kt == KT - 1),
                                )
                        for h2 in range(2):
                            # evict x_aug to SBUF right away so the PSUM bank
                            # frees for the next q-block's AV accumulation
                            xst = rbs_pool.tile(
                                [VW, 512], F32, tag="xst", name=f"xe_{pr}_{qb}_{h2}"
                            )
                            nc.vector.tensor_copy(xst[:], xs[h2][:])
                            rcp = rcp_pool.tile(
                                [1, 512], F32, tag="rcp", name=f"rc_{pr}_{qb}_{h2}"
                            )
                            nc.vector.reciprocal(rcp[:], xst[HD : HD + 1, :])
                            # denominator broadcast across the 64 head dims on
                            # the otherwise-idle GPSIMD engine
                            rb = rbs_pool.tile(
                                [HD, 512], F32, tag="rb", name=f"rb_{pr}_{qb}_{h2}"
                            )
                            nc.gpsimd.partition_broadcast(rb[:], rcp[0:1, :])
                            nc.vector.tensor_tensor(
                                _r(
                                    xtn_sb[
                                        h2 * HD : (h2 + 1) * HD,
                                        pr,
                                        qb * 512 : (qb + 1) * 512,
                                    ]
                                ),
                                xst[0:HD, :],
                                rb[:],
                                op=mybir.AluOpType.mult,
                            )

            # ---------------- Phase C: output projection ----------------
            with (
                tc.tile_pool(name="ostage", bufs=4) as ost_pool,
                tc.tile_pool(name="o_ps", bufs=6, space="PSUM") as o_psum,
            ):
                for qt in range(SQ // P):
                    for ob in range(2):
                        ps = o_psum.tile(
                            [P, 512], F32, tag="op", name=f"op_{qt}_{ob}"
                        )
                        for pr in range(NP):
                            nc.tensor.matmul(
                                ps[:],
                                _r(xtn_sb[:, pr, qt * P : (qt + 1) * P]),
                                _r(wo_sb[:, pr, ob * 512 : (ob + 1) * 512]),
                                start=(pr == 0),
                                stop=(pr == NP - 1),
                            )
                        st = ost_pool.tile(
                            [P, 512], F32, tag="os", name=f"os_{qt}_{ob}"
                        )
                        nc.vector.tensor_tensor(
                            st[:],
                            ps[:],
                            bo_bcast[:, ob * 512 : (ob + 1) * 512],
                            op=mybir.AluOpType.add,
                        )
                        nc.sync.dma_start(
                            out=io["out"][
                                qt * P : (qt + 1) * P, ob * 512 : (ob + 1) * 512
                            ],
                            in_=st[:],
                        )




def build_module(reps=1):
    key = ("nc", reps)
    if key in _CACHE:
        return _CACHE[key]
    nc = bacc.Bacc("TRN2", target_bir_lowering=False, debug=False, num_devices=8)
    io = {}
    io["xq"] = nc.dram_tensor("xq", [SQ, D], F32, kind="ExternalInput").ap()
    io["xk"] = nc.dram_tensor("xk", [SK, D], F32, kind="ExternalInput").ap()
    io["xv"] = nc.dram_tensor("xv", [SK, D], F32, kind="ExternalInput").ap()
    for w in ("wq", "wk", "wv", "wo"):
        io[w] = nc.dram_tensor(w, [D, D], F32, kind="ExternalInput").ap()
    for b in ("bq", "bk", "bv", "bo"):
        io[b] = nc.dram_tensor(b, [D], F32, kind="ExternalInput").ap()
    io["out"] = nc.dram_tensor("out", [SQ, D], F32, kind="ExternalOutput").ap()
    io["qt_dram"] = nc.dram_tensor("qt_scratch", [D, SQ], mybir.dt.bfloat16).ap()
    io["v_dram"] = nc.dram_tensor("v_scratch", [SK, H * VW], mybir.dt.bfloat16).ap()

    with tile.TileContext(nc) as tc:
        for _ in range(reps):
            _emit(tc, io)
    nc.compile()
    _CACHE[key] = nc
    return nc


LAST_RESULTS = None


def kernel(query, key, value, Wq, bq, Wk, bk, Wv, bv, Wo, bo):
    global LAST_RESULTS
    nc = build_module()
    query = np.ascontiguousarray(np.asarray(query, np.float32))
    key = np.ascontiguousarray(np.asarray(key, np.float32))
    value = np.ascontiguousarray(np.asarray(value, np.float32))
    shared = {
        "wq": np.ascontiguousarray(np.asarray(Wq, np.float32)),
        "wk": np.ascontiguousarray(np.asarray(Wk, np.float32)),
        "wv": np.ascontiguousarray(np.asarray(Wv, np.float32)),
        "wo": np.ascontiguousarray(np.asarray(Wo, np.float32)),
        "bq": np.ascontiguousarray(np.asarray(bq, np.float32)),
        "bk": np.ascontiguousarray(np.asarray(bk, np.float32)),
        "bv": np.ascontiguousarray(np.asarray(bv, np.float32)),
        "bo": np.ascontiguousarray(np.asarray(bo, np.float32)),
    }
    in_maps = []
    for c in range(8):
        b, qh = divmod(c, 2)
        in_maps.append(
            {
                "xq": np.ascontiguousarray(query[b, qh * SQ : (qh + 1) * SQ]),
                "xk": key[b],
                "xv": value[b],
                **shared,
            }
        )
    try:
        res = run_bass_kernel_spmd(nc, in_maps, core_ids=list(range(8)))
    except ModuleNotFoundError:
        # BASS_TRACE was requested but this container lacks the axon NTFF
        # profiling hook module; rerun with tracing disabled.
        os.environ["BASS_NEVER_TRACE"] = "1"
        res = run_bass_kernel_spmd(nc, in_maps, core_ids=list(range(8)))
    LAST_RESULTS = res
    out = np.empty((B, S, D), np.float32)
    for c in range(8):
        b, qh = divmod(c, 2)
        out[b, qh * SQ : (qh + 1) * SQ] = res.results[c]["out"]
    return out

